# revision 5
# baseline (speedup 1.0000x reference)
"""Trainium2 Bass kernel for CohereAttention (T=2048, H=4096, NH=32, NKV=8, HD=128).

Sharding: tensor-parallel across heads on 8 cores (SGLang-style).
  - core c owns q-heads [4c, 4c+4) and kv-head c (GQA rep=4 maps exactly).
  - w_qkv column-sharded per core -> [4096, 768] (4q|1k|1v head blocks).
  - attention output (bf16, d-major [512, 2048]) AllGather'd across cores
    (full-block gathers for query blocks 0/1, per-head sub-gathers for 2/3
    so the tail collective is small).
  - w_o column-sharded -> each core computes a [2048, 512] column shard of the
    output (stored transposed [512, 2048]); host concatenates.

Schedule (single emission stream; engine queues execute in program order):
  - startup: minimal DMA set gates the first matmul (~12us), rest streams.
  - P1 tts 0..15: qkv matmuls in 4-ko runs; attention for query blocks 0-2
    interleaved between runs as micro-closures (scores emitted one step ahead
    of the exp->PV chain so the PE never stalls on the ACT engine).
  - ACT engine runs only Exp/Ln (one activation-table set): LN rstd is
    exp(-0.5*ln(var/HD+eps)); all PSUM evictions are DVE copies.
  - post-P1: block 3 attention dense (ACT-bound) with a few o_proj fills,
    then o_proj blocks 0-3 in 8-ko runs; per-head sub-gathers for block 3
    land while o_proj 0-2 compute; tq3 last.
"""

import numpy as np
import ml_dtypes

T = 2048
H = 4096
NH = 32
NKV = 8
HD = 128
N_CORES = 8
QH = NH // N_CORES          # q heads per core = 4
LNH = QH + 1                # layernormed heads per core (4 q + 1 k)
EPS = 1e-5
THETA = 10000.0
SCALE = HD ** -0.5
TT = T // 128               # 16 token tiles
KO = H // 128               # 32 contraction chunks
QC = T // 512               # 4 query chunks of 512
BF16 = ml_dtypes.bfloat16

_CACHE = {}


def _build():
    import concourse.bass as bass
    import concourse.mybir as mybir
    import concourse.tile as tile
    from concourse import bacc
    from contextlib import ExitStack

    dt = mybir.dt
    f32 = dt.float32
    bf16 = dt.bfloat16
    AX = mybir.AxisListType
    OP = mybir.AluOpType
    ACT = mybir.ActivationFunctionType

    nc = bacc.Bacc("TRN2", target_bir_lowering=False, debug=False,
                   num_devices=N_CORES)

    # ---- I/O ----
    hT = nc.dram_tensor("hT", [TT, 128, KO, 128], bf16, kind="ExternalInput")
    wqkv = nc.dram_tensor("wqkv", [H, 768], bf16, kind="ExternalInput")
    wo = nc.dram_tensor("wo", [H, 512], bf16, kind="ExternalInput")
    cosd = nc.dram_tensor("cosd", [128, TT, 64], f32, kind="ExternalInput")
    sind = nc.dram_tensor("sind", [128, TT, 64], f32, kind="ExternalInput")
    lnw = nc.dram_tensor("lnw", [128, LNH, 128], f32, kind="ExternalInput")
    triu = nc.dram_tensor("triu", [128, 128], bf16, kind="ExternalInput")
    ident = nc.dram_tensor("ident", [128, 128], bf16, kind="ExternalInput")
    onesd = nc.dram_tensor("onesd", [128, 128], bf16, kind="ExternalInput")
    outT = nc.dram_tensor("outT", [512, T], f32, kind="ExternalOutput")

    with tile.TileContext(nc) as tc, ExitStack() as ctx:
        const = ctx.enter_context(tc.tile_pool(name="const", bufs=1))
        dram = ctx.enter_context(tc.tile_pool(name="dram", bufs=1, space="DRAM"))

        # DRAM staging for the attention-output gathers.
        #   qc 0/1: whole-block [512, 512] -> [4096, 512] (core-major rows).
        #   qc 2/3: per-head [128, 512] -> [1024, 512] (head h of each core).
        ag_in = [dram.tile([QH * 128, 512], bf16, name=f"agi{q}")
                 for q in range(2)]
        ag_out = [dram.tile([NH * 128, 512], bf16, addr_space="Shared",
                            name=f"ago{q}")
                  for q in range(2)]
        agh_in = {(q, h): dram.tile([128, 512], bf16, name=f"aghi{q}_{h}")
                  for q in (2, 3) for h in range(QH)}
        agh_out = {(q, h): dram.tile([N_CORES * 128, 512], bf16,
                                     addr_space="Shared", name=f"agho{q}_{h}")
                   for q in (2, 3) for h in range(QH)}

        with tc.tile_pool(name="sps", bufs=2, space="PSUM") as sps, \
             tc.tile_pool(name="pvsm", bufs=1, space="PSUM") as pvsm, \
             tc.tile_pool(name="probs", bufs=4) as probs, \
             tc.tile_pool(name="attn", bufs=2) as attn, \
             tc.tile_pool(name="acts", bufs=1) as acts:

            # persistent activations: d-major Q/K, t-major V (bf16)
            QT = acts.tile([128, QH, TT, 128], bf16)    # [d, h, tt, t]
            KT = acts.tile([128, TT, 128], bf16)        # [d, kt, t]
            Vt = acts.tile([128, TT, 128], bf16)        # [t, kt, d]

            # consts declared early (DMAs emitted in priority order below)
            cos_sb = const.tile([128, TT, 64], f32)
            sin_sb = const.tile([128, TT, 64], f32)
            lnw_sb = const.tile([128, LNH, 128], f32)
            triu_sb = const.tile([128, 128], bf16)
            ident_sb = const.tile([128, 128], bf16)
            ones_sb = const.tile([128, 128], bf16)
            wo_sb = const.tile([128, KO, 512], bf16)
            eps_sb = const.tile([128, 1], f32)
            nc.vector.memset(eps_sb[:], EPS)

            # ---- attention micro-closures (software-pipelined per head) ----
            hstate = {}

            def emit_ss(qc, h, kt):
                m = max(kt - 4 * qc, 0)
                lo = m * 128
                ss = sps.tile([128, 512], f32, tag="ss")
                nc.tensor.matmul(ss[:, lo:512], KT[:, kt, :],
                                 QT[:, h, 4 * qc + m:4 * qc + 4, :],
                                 start=True, stop=True)
                return (ss, m, lo)

            def prime(qc, h):
                st = {'pvs': pvsm.tile([128, 1024], f32, tag="pvsm",
                                       name="pvs"),
                      'ss': {}}
                nkt = 4 * (qc + 1)
                for k in (0, 1):
                    if k < nkt:
                        st['ss'][k] = emit_ss(qc, h, k)
                hstate[(qc, h)] = st

            def finish(qc, h, kt):
                st = hstate[(qc, h)]
                nkt = 4 * (qc + 1)
                ss, m, lo = st['ss'].pop(kt)
                pvs = st['pvs']
                pv = pvs[:, 0:512]
                sm = pvs[:, 512:1024]
                pT = probs.tile([128, 4, 128], bf16, tag="pT")
                pTf = pT.rearrange("p a b -> p (a b)")
                nc.scalar.activation(pTf[:, lo:512], ss[:, lo:512],
                                     ACT.Exp, scale=SCALE)
                if kt >= 4 * qc:
                    nc.vector.tensor_tensor(pT[:, m, :], pT[:, m, :],
                                            triu_sb[:], OP.mult)
                nc.tensor.matmul(pv[:, lo:512], Vt[:, kt, :], pTf[:, lo:512],
                                 start=(kt == 0), stop=(kt == nkt - 1))
                nc.tensor.matmul(sm[:, lo:512], ones_sb[:], pTf[:, lo:512],
                                 start=(kt == 0), stop=(kt == nkt - 1))
                if kt + 2 < nkt:
                    st['ss'][kt + 2] = emit_ss(qc, h, kt + 2)

            def finalize(qc, h):
                st = hstate.pop((qc, h))
                pvs = st['pvs']
                pvsb = attn.tile([128, 1024], f32, tag="pvsb")
                nc.vector.tensor_copy(pvsb[:], pvs[:])
                recip = attn.tile([128, 512], f32, tag="recip")
                nc.vector.reciprocal_approx_fast(recip[:], pvsb[:, 512:1024])
                at = attn.tile([128, 512], bf16, tag="at")
                nc.vector.tensor_tensor(at[:], pvsb[:, 0:512], recip[:],
                                        OP.mult)
                if qc <= 1:
                    nc.sync.dma_start(ag_in[qc][h * 128:(h + 1) * 128, :],
                                      at[:])
                    if h == QH - 1:
                        nc.gpsimd.collective_compute(
                            "AllGather", mybir.AluOpType.bypass,
                            replica_groups=[list(range(N_CORES))],
                            ins=[ag_in[qc].opt()], outs=[ag_out[qc].opt()])
                else:
                    nc.sync.dma_start(agh_in[(qc, h)][:], at[:])
                    nc.gpsimd.collective_compute(
                        "AllGather", mybir.AluOpType.bypass,
                        replica_groups=[list(range(N_CORES))],
                        ins=[agh_in[(qc, h)].opt()],
                        outs=[agh_out[(qc, h)].opt()])

            def block_closures(qc):
                cls = []
                nkt = 4 * (qc + 1)
                for h in range(QH):
                    cls.append(lambda qc=qc, h=h: prime(qc, h))
                    for kt in range(nkt):
                        cls.append(lambda qc=qc, h=h, kt=kt:
                                   finish(qc, h, kt))
                    cls.append(lambda qc=qc, h=h: finalize(qc, h))
                return cls

            from collections import deque
            pending = deque()

            def pump(n):
                for _ in range(n):
                    if pending:
                        pending.popleft()()

            # ---- P1: qkv projection + LN + RoPE + transposes, attention
            # blocks 0-2 interleaved between 4-ko matmul runs ----
            with tc.tile_pool(name="htp", bufs=3) as htp, \
                 tc.tile_pool(name="qkps", bufs=2, space="PSUM") as qkps, \
                 tc.tile_pool(name="p1t", bufs=2) as p1t:

                wqkv_r = wqkv.ap().rearrange("(ko p) n -> p ko n", p=128)
                wqkv_sb = htp.tile([128, KO, 768], bf16, tag="wqkv", bufs=1)

                ht_tiles = {}

                def dma_ht(t):
                    ht_t = htp.tile([128, KO, 128], bf16, tag="ht")
                    ht_tiles[t] = ht_t
                    for c in range(8):
                        nc.sync.dma_start(ht_t[:, 4 * c:4 * (c + 1), :],
                                          hT.ap()[t][:, 4 * c:4 * (c + 1), :])

                # priority DMA order: first wqkv slice + first ht chunks gate
                # the first matmul; everything else streams behind them.
                nc.sync.dma_start(wqkv_sb[:, 0:4, :], wqkv_r[:, 0:4, :])
                dma_ht(0)
                for c in range(1, 8):
                    nc.sync.dma_start(wqkv_sb[:, 4 * c:4 * (c + 1), :],
                                      wqkv_r[:, 4 * c:4 * (c + 1), :])
                nc.sync.dma_start(ident_sb[:], ident.ap())
                nc.sync.dma_start(cos_sb[:], cosd.ap())
                nc.sync.dma_start(sin_sb[:], sind.ap())
                nc.sync.dma_start(lnw_sb[:], lnw.ap())
                dma_ht(1)
                nc.sync.dma_start(triu_sb[:], triu.ap())
                nc.sync.dma_start(ones_sb[:], onesd.ap())
                wo_r = wo.ap().rearrange("(ko p) n -> p ko n", p=128)
                for c in range(8):
                    nc.sync.dma_start(wo_sb[:, 4 * c:4 * (c + 1), :],
                                      wo_r[:, 4 * c:4 * (c + 1), :])

                for tt in range(TT):
                    if tt + 2 < TT:
                        dma_ht(tt + 2)
                    ht_t = ht_tiles.pop(tt)
                    ps = qkps.tile([128, 768], f32, tag="qk")
                    for run in range(8):          # 4-ko matmul runs
                        for ko in range(4 * run, 4 * run + 4):
                            nc.tensor.matmul(ps[:, 0:512], ht_t[:, ko, :],
                                             wqkv_sb[:, ko, 0:512],
                                             start=(ko == 0),
                                             stop=(ko == KO - 1))
                            nc.tensor.matmul(ps[:, 512:768], ht_t[:, ko, :],
                                             wqkv_sb[:, ko, 512:768],
                                             start=(ko == 0),
                                             stop=(ko == KO - 1))
                        pump(1)

                    qkv_t = p1t.tile([128, 768], f32, tag="qkv")
                    nc.vector.tensor_copy(qkv_t[:], ps[:])

                    # V: plain bf16 cast into persistent tile
                    nc.vector.tensor_copy(Vt[:, tt, :], qkv_t[:, 640:768])

                    # layernorm over the 5 q/k heads (ACT: Ln+Exp only)
                    x5 = qkv_t[:, 0:640].rearrange("p (h d) -> p h d", d=128)
                    mean = p1t.tile([128, LNH], f32, tag="mean")
                    nc.vector.tensor_reduce(mean[:], x5, AX.X, OP.add)
                    nc.vector.tensor_scalar_mul(mean[:], mean[:], 1.0 / HD)
                    xc = p1t.tile([128, LNH, 128], f32, tag="xc")
                    nc.vector.tensor_tensor(
                        xc[:], x5, mean[:, :, None].to_broadcast((128, LNH, 128)),
                        OP.subtract)
                    sq = p1t.tile([128, LNH, 128], f32, tag="sq")
                    nc.vector.tensor_tensor(sq[:], xc[:], xc[:], OP.mult)
                    var = p1t.tile([128, LNH], f32, tag="var")
                    nc.vector.tensor_reduce(var[:], sq[:], AX.X, OP.add)
                    # rstd = exp(-0.5 * ln(var/HD + eps)); Ln and Exp share an
                    # activation-table set, so no table reloads vs Sqrt.
                    lnv = p1t.tile([128, LNH], f32, tag="lnv")
                    nc.scalar.activation(lnv[:], var[:], ACT.Ln,
                                         bias=eps_sb[:], scale=1.0 / HD)
                    rstd = p1t.tile([128, LNH], f32, tag="rstd")
                    nc.scalar.activation(rstd[:], lnv[:], ACT.Exp, scale=-0.5)
                    nc.vector.tensor_tensor(
                        xc[:], xc[:], rstd[:, :, None].to_broadcast((128, LNH, 128)),
                        OP.mult)
                    nc.vector.tensor_tensor(xc[:], xc[:], lnw_sb[:], OP.mult)

                    # interleaved RoPE
                    x1 = xc[:, :, 0:128:2]
                    x2 = xc[:, :, 1:128:2]
                    cos_b = cos_sb[:, tt:tt + 1, :].to_broadcast((128, LNH, 64))
                    sin_b = sin_sb[:, tt:tt + 1, :].to_broadcast((128, LNH, 64))
                    m1 = p1t.tile([128, LNH, 64], f32, tag="m1")
                    m2 = p1t.tile([128, LNH, 64], f32, tag="m2")
                    qkf = p1t.tile([128, LNH, 128], bf16, tag="qkf")
                    nc.vector.tensor_tensor(m1[:], x1, cos_b, OP.mult)
                    nc.vector.tensor_tensor(m2[:], x2, sin_b, OP.mult)
                    nc.vector.tensor_tensor(qkf[:, :, 0:128:2], m1[:], m2[:],
                                            OP.subtract)
                    nc.vector.tensor_tensor(m1[:], x2, cos_b, OP.mult)
                    nc.vector.tensor_tensor(m2[:], x1, sin_b, OP.mult)
                    nc.vector.tensor_tensor(qkf[:, :, 1:128:2], m1[:], m2[:],
                                            OP.add)

                    # transpose each head tile [t,d] -> [d,t]
                    for h5 in range(LNH):
                        pst = sps.tile([128, 128], bf16, tag="ss")
                        nc.tensor.transpose(pst[:], qkf[:, h5, :], ident_sb[:])
                        if h5 < QH:
                            nc.vector.tensor_copy(QT[:, h5, tt, :], pst[:])
                        else:
                            nc.vector.tensor_copy(KT[:, tt, :], pst[:])
                    pump(1)

                    if tt % 4 == 3 and tt < TT - 1:
                        pending.extend(block_closures(tt // 4))

                # drain any leftover block-2 closures before P1 pools close
                while pending:
                    pending.popleft()()

            # ---- post-P1: block 3 dense + o_proj in chunks ----
            with tc.tile_pool(name="agp", bufs=2) as agp, \
                 tc.tile_pool(name="ops", bufs=2, space="PSUM") as ops, \
                 tc.tile_pool(name="osb", bufs=2) as osb:

                rt_tiles = {}
                po_tiles = {}

                def oproj_load(tq):
                    rt = agp.tile([128, KO, 512], bf16, tag="rt")
                    rt_tiles[tq] = rt
                    if tq <= 1:
                        agr = ag_out[tq].rearrange("(ko p) n -> p ko n", p=128)
                        for c in range(8):
                            nc.sync.dma_start(rt[:, 4 * c:4 * (c + 1), :],
                                              agr[:, 4 * c:4 * (c + 1), :])
                    else:
                        # per-head gather layout: rt ko-chunk 4c+h comes from
                        # rows c*128 of agh_out[(tq, h)]
                        rt_r = rt.rearrange("p (c h) n -> p c h n", h=QH)
                        for h in range(QH):
                            src = agh_out[(tq, h)].rearrange(
                                "(c p) n -> p c n", p=128)
                            nc.sync.dma_start(rt_r[:, :, h, :], src)

                def oproj_run(tq, hc, r):
                    rt = rt_tiles[tq]
                    if r == 0:
                        po_tiles[(tq, hc)] = ops.tile([128, 512], f32,
                                                      tag="po", name="po")
                    po = po_tiles[(tq, hc)]
                    for ko in range(8 * r, 8 * r + 8):
                        nc.tensor.matmul(po[:],
                                         wo_sb[:, ko, hc * 128:(hc + 1) * 128],
                                         rt[:, ko, :],
                                         start=(ko == 0), stop=(ko == KO - 1))

                def oproj_fin(tq, hc):
                    po = po_tiles.pop((tq, hc))
                    ot = osb.tile([128, 512], f32, tag="ot")
                    nc.vector.tensor_copy(ot[:], po[:])
                    nc.sync.dma_start(
                        outT.ap()[hc * 128:(hc + 1) * 128,
                                  tq * 512:(tq + 1) * 512],
                        ot[:])

                def oproj_chunks(tq, with_load=True):
                    cls = []
                    if with_load:
                        cls.append(lambda tq=tq: oproj_load(tq))
                    for hc in range(4):
                        for r in range(4):
                            cls.append(lambda tq=tq, hc=hc, r=r:
                                       oproj_run(tq, hc, r))
                        cls.append(lambda tq=tq, hc=hc: oproj_fin(tq, hc))
                    return cls

                fillers = deque(oproj_chunks(0) + oproj_chunks(1))

                # block 3: dense (ACT-bound); a filler o_proj run every few
                # steps soaks the small PE slack without delaying the exps.
                b3 = block_closures(3)
                # rt0 load first: its gather completed long ago
                fillers.popleft()()
                for i, cl in enumerate(b3):
                    cl()
                    if i % 8 == 7 and fillers:
                        fillers.popleft()()
                while fillers:
                    fillers.popleft()()

                for cl in oproj_chunks(2):
                    cl()
                for cl in oproj_chunks(3):
                    cl()

    nc.compile()
    return nc


def _prep_inputs(positions, hidden_states, w_qkv, w_o, q_norm_w, k_norm_w):
    hidden_states = np.asarray(hidden_states, dtype=np.float32)
    w_qkv = np.asarray(w_qkv, dtype=np.float32)
    w_o = np.asarray(w_o, dtype=np.float32)
    q_norm_w = np.asarray(q_norm_w, dtype=np.float32)
    k_norm_w = np.asarray(k_norm_w, dtype=np.float32)
    pos = np.asarray(positions).astype(np.float32)

    # hiddenT tiled for contiguous per-partition DMA: [tt, p, ko, tl]
    hT = np.ascontiguousarray(
        hidden_states.reshape(TT, 128, KO, 128).transpose(0, 3, 2, 1)
    ).astype(BF16)

    inv_freq = THETA ** (-np.arange(64, dtype=np.float32) / 64.0)
    freqs = pos[:, None] * inv_freq[None, :]
    cos = np.cos(freqs).astype(np.float32).reshape(TT, 128, 64).transpose(1, 0, 2)
    sin = np.sin(freqs).astype(np.float32).reshape(TT, 128, 64).transpose(1, 0, 2)
    cos = np.ascontiguousarray(cos)
    sin = np.ascontiguousarray(sin)

    triu = np.triu(np.ones((128, 128), dtype=np.float32)).astype(BF16)
    identm = np.eye(128, dtype=np.float32).astype(BF16)
    onesm = np.ones((128, 128), dtype=np.float32).astype(BF16)

    in_maps = []
    for c in range(N_CORES):
        qcols = w_qkv[:, 4 * c * HD:(4 * c + 4) * HD]
        kcols = w_qkv[:, NH * HD + c * HD: NH * HD + (c + 1) * HD]
        vcols = w_qkv[:, (NH + NKV) * HD + c * HD: (NH + NKV) * HD + (c + 1) * HD]
        wqkv_sh = np.concatenate([qcols, kcols, vcols], axis=1).astype(BF16)
        wo_sh = np.ascontiguousarray(w_o[:, 512 * c:512 * (c + 1)]).astype(BF16)
        ln5 = np.concatenate([q_norm_w[4 * c:4 * c + 4], k_norm_w[c:c + 1]], axis=0)
        lnw_rep = np.ascontiguousarray(
            np.broadcast_to(ln5[None, :, :], (128, LNH, 128))).astype(np.float32)
        in_maps.append({
            "hT": hT,
            "wqkv": wqkv_sh,
            "wo": wo_sh,
            "cosd": cos,
            "sind": sin,
            "lnw": lnw_rep,
            "triu": triu,
            "ident": identm,
            "onesd": onesm,
        })
    return in_maps


def kernel(positions, hidden_states, w_qkv, w_o, q_norm_w, k_norm_w):
    from concourse.bass_utils import run_bass_kernel_spmd

    if "nc" not in _CACHE:
        _CACHE["nc"] = _build()
    nc = _CACHE["nc"]

    in_maps = _prep_inputs(positions, hidden_states, w_qkv, w_o,
                           q_norm_w, k_norm_w)
    res = run_bass_kernel_spmd(nc, in_maps, core_ids=list(range(N_CORES)))
    out = np.empty((T, H), dtype=np.float32)
    for c in range(N_CORES):
        out[:, 512 * c:512 * (c + 1)] = res.results[c]["outT"].T
    return out


# revision 7
# speedup vs baseline: 1.0851x; 1.0851x over previous
"""Trainium2 Bass kernel for CohereAttention (T=2048, H=4096, NH=32, NKV=8, HD=128).

Sharding: tensor-parallel across heads on 8 cores (SGLang-style).
  - core c owns q-heads [4c, 4c+4) and kv-head c (GQA rep=4 maps exactly).
  - w_qkv column-sharded per core -> [4096, 768] (4q|1k|1v head blocks).
  - attention output (bf16, d-major [512, 2048]) AllGather'd across cores
    (full-block gathers for query blocks 0-2, per-head sub-gathers for 3
    so the tail collective drains in small pieces).
  - w_o column-sharded -> each core computes a [2048, 512] column shard of the
    output (stored transposed [512, 2048]); host concatenates.

Schedule (single emission stream; engine queues execute in program order):
  - startup: minimal DMA set gates the first matmul (~12us), rest streams.
  - P1 tts 0..15: qkv matmuls in 4-ko runs; attention for query blocks 0-2
    interleaved between runs as micro-closures (scores emitted one step ahead
    of the exp->PV chain so the PE never stalls on the ACT engine).
  - ACT engine runs ONLY Exp (one table set, zero reloads): LN rstd is
    computed on the DVE as pow(var/HD+eps, -0.5); PSUM evictions are DVE.
  - post-P1: block 3 attention dense (ACT-bound) with o_proj fills, then
    o_proj blocks 0-3 in 8-ko runs; block 3 per-head sub-gathers land while
    o_proj 0-2 compute; tq3 last.
"""

import numpy as np
import ml_dtypes

T = 2048
H = 4096
NH = 32
NKV = 8
HD = 128
N_CORES = 8
QH = NH // N_CORES          # q heads per core = 4
LNH = QH + 1                # layernormed heads per core (4 q + 1 k)
EPS = 1e-5
THETA = 10000.0
SCALE = HD ** -0.5
TT = T // 128               # 16 token tiles
KO = H // 128               # 32 contraction chunks
QC = T // 512               # 4 query chunks of 512
BF16 = ml_dtypes.bfloat16

_CACHE = {}


def _build():
    import concourse.bass as bass
    import concourse.mybir as mybir
    import concourse.tile as tile
    from concourse import bacc
    from contextlib import ExitStack
    from collections import deque
    import math

    dt = mybir.dt
    f32 = dt.float32
    bf16 = dt.bfloat16
    AX = mybir.AxisListType
    OP = mybir.AluOpType
    ACT = mybir.ActivationFunctionType

    nc = bacc.Bacc("TRN2", target_bir_lowering=False, debug=False,
                   num_devices=N_CORES)

    # ---- I/O ----
    hT = nc.dram_tensor("hT", [TT, 128, KO, 128], bf16, kind="ExternalInput")
    wqkv = nc.dram_tensor("wqkv", [H, 768], bf16, kind="ExternalInput")
    wo = nc.dram_tensor("wo", [H, 512], bf16, kind="ExternalInput")
    cosd = nc.dram_tensor("cosd", [128, TT, 64], f32, kind="ExternalInput")
    sind = nc.dram_tensor("sind", [128, TT, 64], f32, kind="ExternalInput")
    lnw = nc.dram_tensor("lnw", [128, LNH, 128], f32, kind="ExternalInput")
    triu = nc.dram_tensor("triu", [128, 128], bf16, kind="ExternalInput")
    ident = nc.dram_tensor("ident", [128, 128], bf16, kind="ExternalInput")
    onesd = nc.dram_tensor("onesd", [128, 128], bf16, kind="ExternalInput")
    outT = nc.dram_tensor("outT", [512, T], f32, kind="ExternalOutput")

    with tile.TileContext(nc) as tc, ExitStack() as ctx:
        const = ctx.enter_context(tc.tile_pool(name="const", bufs=1))
        dram = ctx.enter_context(tc.tile_pool(name="dram", bufs=1, space="DRAM"))

        # DRAM staging for the attention-output gathers.
        #   qc 0-2: whole-block [512, 512] -> [4096, 512] (core-major rows).
        #   qc 3: per-head [128, 512] -> [1024, 512] (head h of each core).
        ag_in = [dram.tile([QH * 128, 512], bf16, name=f"agi{q}")
                 for q in range(3)]
        ag_out = [dram.tile([NH * 128, 512], bf16, addr_space="Shared",
                            name=f"ago{q}")
                  for q in range(3)]
        agh_in = [dram.tile([128, 512], bf16, name=f"aghi{h}")
                  for h in range(QH)]
        agh_out = [dram.tile([N_CORES * 128, 512], bf16, addr_space="Shared",
                             name=f"agho{h}")
                   for h in range(QH)]

        with tc.tile_pool(name="sps", bufs=2, space="PSUM") as sps, \
             tc.tile_pool(name="pvsm", bufs=2, space="PSUM") as pvsm, \
             tc.tile_pool(name="probs", bufs=4) as probs, \
             tc.tile_pool(name="attn", bufs=2) as attn, \
             tc.tile_pool(name="acts", bufs=1) as acts:

            # persistent activations: d-major Q/K, t-major V (bf16)
            QT = acts.tile([128, QH, TT, 128], bf16)    # [d, h, tt, t]
            KT = acts.tile([128, TT, 128], bf16)        # [d, kt, t]
            Vt = acts.tile([128, TT, 128], bf16)        # [t, kt, d]

            cos_sb = const.tile([128, TT, 64], f32)
            sin_sb = const.tile([128, TT, 64], f32)
            lnw_sb = const.tile([128, LNH, 128], f32)
            triu_sb = const.tile([128, 128], bf16)
            ident_sb = const.tile([128, 128], bf16)
            ones_sb = const.tile([128, 128], bf16)
            wo_sb = const.tile([128, KO, 512], bf16)

            # ---- attention micro-closures (software-pipelined per head) ----
            hstate = {}

            def emit_ss(qc, h, kt):
                m = max(kt - 4 * qc, 0)
                lo = m * 128
                ss = sps.tile([128, 512], f32, tag="ss", name="ss")
                nc.tensor.matmul(ss[:, lo:512], KT[:, kt, :],
                                 QT[:, h, 4 * qc + m:4 * qc + 4, :],
                                 start=True, stop=True)
                return (ss, m, lo)

            def prime(qc, h):
                st = {'pvs': pvsm.tile([128, 1024], f32, tag="pvsm",
                                       name="pvs"),
                      'ss': {}}
                nkt = 4 * (qc + 1)
                for k in (0, 1):
                    if k < nkt:
                        st['ss'][k] = emit_ss(qc, h, k)
                hstate[(qc, h)] = st

            def finish(qc, h, kt):
                st = hstate[(qc, h)]
                nkt = 4 * (qc + 1)
                ss, m, lo = st['ss'].pop(kt)
                pvs = st['pvs']
                pv = pvs[:, 0:512]
                sm = pvs[:, 512:1024]
                pT = probs.tile([128, 4, 128], bf16, tag="pT")
                pTf = pT.rearrange("p a b -> p (a b)")
                nc.scalar.activation(pTf[:, lo:512], ss[:, lo:512],
                                     ACT.Exp, scale=SCALE)
                if kt >= 4 * qc:
                    nc.vector.tensor_tensor(pT[:, m, :], pT[:, m, :],
                                            triu_sb[:], OP.mult)
                nc.tensor.matmul(pv[:, lo:512], Vt[:, kt, :], pTf[:, lo:512],
                                 start=(kt == 0), stop=(kt == nkt - 1))
                nc.tensor.matmul(sm[:, lo:512], ones_sb[:], pTf[:, lo:512],
                                 start=(kt == 0), stop=(kt == nkt - 1))
                if kt + 2 < nkt:
                    st['ss'][kt + 2] = emit_ss(qc, h, kt + 2)

            def finalize(qc, h):
                st = hstate.pop((qc, h))
                pvs = st['pvs']
                # split evictions: each half frees its psum region separately
                pvc = attn.tile([128, 512], f32, tag="pvc")
                nc.vector.tensor_copy(pvc[:], pvs[:, 0:512])
                smc = attn.tile([128, 512], f32, tag="smc")
                nc.vector.tensor_copy(smc[:], pvs[:, 512:1024])
                recip = attn.tile([128, 512], f32, tag="recip")
                nc.vector.reciprocal_approx_fast(recip[:], smc[:])
                at = attn.tile([128, 512], bf16, tag="at")
                nc.vector.tensor_tensor(at[:], pvc[:], recip[:], OP.mult)
                if qc <= 2:
                    nc.sync.dma_start(ag_in[qc][h * 128:(h + 1) * 128, :],
                                      at[:])
                    if h == QH - 1:
                        nc.gpsimd.collective_compute(
                            "AllGather", mybir.AluOpType.bypass,
                            replica_groups=[list(range(N_CORES))],
                            ins=[ag_in[qc].opt()], outs=[ag_out[qc].opt()])
                else:
                    nc.sync.dma_start(agh_in[h][:], at[:])
                    nc.gpsimd.collective_compute(
                        "AllGather", mybir.AluOpType.bypass,
                        replica_groups=[list(range(N_CORES))],
                        ins=[agh_in[h].opt()], outs=[agh_out[h].opt()])

            def block_closures(qc):
                cls = []
                nkt = 4 * (qc + 1)
                for h in range(QH):
                    cls.append(lambda qc=qc, h=h: prime(qc, h))
                    for kt in range(nkt):
                        cls.append(lambda qc=qc, h=h, kt=kt:
                                   finish(qc, h, kt))
                    cls.append(lambda qc=qc, h=h: finalize(qc, h))
                return cls

            pending = deque()

            def pump():
                n = min(4, max(1, math.ceil(len(pending) / 18)))
                for _ in range(n):
                    if pending:
                        pending.popleft()()

            # ---- P1: qkv projection + LN + RoPE + transposes, attention
            # blocks 0-2 interleaved between 4-ko matmul runs ----
            with tc.tile_pool(name="htp", bufs=3) as htp, \
                 tc.tile_pool(name="qkps", bufs=1, space="PSUM") as qkps, \
                 tc.tile_pool(name="p1t", bufs=2) as p1t:

                wqkv_r = wqkv.ap().rearrange("(ko p) n -> p ko n", p=128)
                wqkv_sb = htp.tile([128, KO, 768], bf16, tag="wqkv", bufs=1)

                ht_tiles = {}

                def dma_ht(t):
                    ht_t = htp.tile([128, KO, 128], bf16, tag="ht",
                                    name="ht_t")
                    ht_tiles[t] = ht_t
                    for c in range(8):
                        nc.sync.dma_start(ht_t[:, 4 * c:4 * (c + 1), :],
                                          hT.ap()[t][:, 4 * c:4 * (c + 1), :])

                # priority DMA order: first wqkv slice + first ht chunks gate
                # the first matmul; everything else streams behind them.
                nc.sync.dma_start(wqkv_sb[:, 0:4, :], wqkv_r[:, 0:4, :])
                dma_ht(0)
                for c in range(1, 8):
                    nc.sync.dma_start(wqkv_sb[:, 4 * c:4 * (c + 1), :],
                                      wqkv_r[:, 4 * c:4 * (c + 1), :])
                nc.sync.dma_start(ident_sb[:], ident.ap())
                nc.sync.dma_start(cos_sb[:], cosd.ap())
                nc.sync.dma_start(sin_sb[:], sind.ap())
                nc.sync.dma_start(lnw_sb[:], lnw.ap())
                dma_ht(1)
                nc.sync.dma_start(triu_sb[:], triu.ap())
                nc.sync.dma_start(ones_sb[:], onesd.ap())
                wo_r = wo.ap().rearrange("(ko p) n -> p ko n", p=128)
                for c in range(8):
                    nc.sync.dma_start(wo_sb[:, 4 * c:4 * (c + 1), :],
                                      wo_r[:, 4 * c:4 * (c + 1), :])

                for tt in range(TT):
                    if tt + 2 < TT:
                        dma_ht(tt + 2)
                    ht_t = ht_tiles.pop(tt)
                    ps = qkps.tile([128, 768], f32, tag="qk")
                    # q-half first (all 32 ko), then kv-half: the q eviction
                    # overlaps the kv matmuls, so qkps bufs=1 stalls ~nothing
                    for run in range(8):
                        for ko in range(4 * run, 4 * run + 4):
                            nc.tensor.matmul(ps[:, 0:512], ht_t[:, ko, :],
                                             wqkv_sb[:, ko, 0:512],
                                             start=(ko == 0),
                                             stop=(ko == KO - 1))
                        pump()
                    qkv_q = p1t.tile([128, 512], f32, tag="qkv_q")
                    nc.vector.tensor_copy(qkv_q[:], ps[:, 0:512])
                    for run in range(8):
                        for ko in range(4 * run, 4 * run + 4):
                            nc.tensor.matmul(ps[:, 512:768], ht_t[:, ko, :],
                                             wqkv_sb[:, ko, 512:768],
                                             start=(ko == 0),
                                             stop=(ko == KO - 1))
                        pump()
                    qkv_kv = p1t.tile([128, 256], f32, tag="qkv_kv")
                    nc.vector.tensor_copy(qkv_kv[:], ps[:, 512:768])

                    # V: plain bf16 cast into persistent tile
                    nc.vector.tensor_copy(Vt[:, tt, :], qkv_kv[:, 128:256])

                    # layernorm over the 5 q/k heads (no ACT engine use)
                    x5q = qkv_q.rearrange("p (h d) -> p h d", d=128)
                    x5k = qkv_kv[:, 0:128]
                    mean = p1t.tile([128, LNH], f32, tag="mean")
                    nc.vector.tensor_reduce(mean[:, 0:QH], x5q, AX.X, OP.add)
                    nc.vector.tensor_reduce(mean[:, QH:LNH],
                                            qkv_kv[:, None, 0:128], AX.X,
                                            OP.add)
                    nc.vector.tensor_scalar_mul(mean[:], mean[:], 1.0 / HD)
                    xc = p1t.tile([128, LNH, 128], f32, tag="xc")
                    nc.vector.tensor_tensor(
                        xc[:, 0:QH, :], x5q,
                        mean[:, 0:QH, None].to_broadcast((128, QH, 128)),
                        OP.subtract)
                    nc.vector.tensor_tensor(
                        xc[:, QH, :], x5k,
                        mean[:, QH:LNH].to_broadcast((128, 128)),
                        OP.subtract)
                    sq = p1t.tile([128, LNH, 128], f32, tag="sq")
                    nc.vector.tensor_tensor(sq[:], xc[:], xc[:], OP.mult)
                    var = p1t.tile([128, LNH], f32, tag="var")
                    nc.vector.tensor_reduce(var[:], sq[:], AX.X, OP.add)
                    # rstd = (var/HD + eps) ** -0.5 without Sqrt/Ln tables:
                    # y0 ~= ln(v) from the fp32 exponent bits, one exp-Newton
                    # refinement (y1 = y0 + v*exp(-y0) - 1), rstd = exp(-.5*y1).
                    # Uses only the Exp table (shared with attention) -> no
                    # activation-table reloads anywhere in the kernel.
                    vv = p1t.tile([128, LNH], f32, tag="vv")
                    nc.vector.tensor_scalar(vv[:], var[:], 1.0 / HD, EPS,
                                            OP.mult, OP.add)
                    y0 = p1t.tile([128, LNH], f32, tag="y0")
                    nc.vector.tensor_copy(y0[:], vv.bitcast(dt.int32)[:])
                    nc.vector.tensor_scalar(y0[:], y0[:], 8.262958405176314e-08,
                                            -88.02969193111305,
                                            OP.mult, OP.add)
                    ey = p1t.tile([128, LNH], f32, tag="ey")
                    nc.scalar.activation(ey[:], y0[:], ACT.Exp, scale=-1.0)
                    nc.vector.tensor_tensor(ey[:], ey[:], vv[:], OP.mult)
                    nc.vector.tensor_scalar_add(ey[:], ey[:], -1.0)
                    nc.vector.tensor_tensor(y0[:], y0[:], ey[:], OP.add)
                    rstd = p1t.tile([128, LNH], f32, tag="rstd")
                    nc.scalar.activation(rstd[:], y0[:], ACT.Exp, scale=-0.5)
                    nc.vector.tensor_tensor(
                        xc[:], xc[:],
                        rstd[:, :, None].to_broadcast((128, LNH, 128)),
                        OP.mult)
                    nc.vector.tensor_tensor(xc[:], xc[:], lnw_sb[:], OP.mult)

                    # interleaved RoPE
                    x1 = xc[:, :, 0:128:2]
                    x2 = xc[:, :, 1:128:2]
                    cos_b = cos_sb[:, tt:tt + 1, :].to_broadcast((128, LNH, 64))
                    sin_b = sin_sb[:, tt:tt + 1, :].to_broadcast((128, LNH, 64))
                    m1 = p1t.tile([128, LNH, 64], f32, tag="m1")
                    m2 = p1t.tile([128, LNH, 64], f32, tag="m2")
                    qkf = p1t.tile([128, LNH, 128], bf16, tag="qkf")
                    nc.vector.tensor_tensor(m1[:], x1, cos_b, OP.mult)
                    nc.vector.tensor_tensor(m2[:], x2, sin_b, OP.mult)
                    nc.vector.tensor_tensor(qkf[:, :, 0:128:2], m1[:], m2[:],
                                            OP.subtract)
                    nc.vector.tensor_tensor(m1[:], x2, cos_b, OP.mult)
                    nc.vector.tensor_tensor(m2[:], x1, sin_b, OP.mult)
                    nc.vector.tensor_tensor(qkf[:, :, 1:128:2], m1[:], m2[:],
                                            OP.add)

                    # transpose head tiles [t,d]->[d,t] into ONE packed psum
                    # slot (one WAR dep + two DVE copies instead of five)
                    pst = sps.tile([128, LNH, 128], bf16, tag="ss",
                                   name="pst")
                    for h5 in range(LNH):
                        nc.tensor.transpose(pst[:, h5, :], qkf[:, h5, :],
                                            ident_sb[:])
                    nc.vector.tensor_copy(QT[:, :, tt, :], pst[:, 0:QH, :])
                    nc.vector.tensor_copy(KT[:, tt, :], pst[:, QH, :])
                    pump()

                    if tt % 4 == 3 and tt < TT - 1:
                        pending.extend(block_closures(tt // 4))

                # drain any leftover block-2 closures before P1 pools close
                while pending:
                    pending.popleft()()

            # ---- post-P1: block 3 dense + o_proj in chunks ----
            with tc.tile_pool(name="agp", bufs=2) as agp, \
                 tc.tile_pool(name="ops", bufs=2, space="PSUM") as ops, \
                 tc.tile_pool(name="osb", bufs=2) as osb:

                rt_tiles = {}
                po_tiles = {}

                def oproj_load(tq):
                    rt = agp.tile([128, KO, 512], bf16, tag="rt", name="rt")
                    rt_tiles[tq] = rt
                    if tq <= 2:
                        agr = ag_out[tq].rearrange("(ko p) n -> p ko n", p=128)
                        for c in range(8):
                            nc.sync.dma_start(rt[:, 4 * c:4 * (c + 1), :],
                                              agr[:, 4 * c:4 * (c + 1), :])
                    else:
                        # per-head gather layout: rt ko-chunk 4c+h comes from
                        # rows c*128 of agh_out[h]
                        rt_r = rt.rearrange("p (c h) n -> p c h n", h=QH)
                        for h in range(QH):
                            src = agh_out[h].rearrange("(c p) n -> p c n",
                                                       p=128)
                            nc.sync.dma_start(rt_r[:, :, h, :], src)

                def oproj_run(tq, hc, r):
                    rt = rt_tiles[tq]
                    if r == 0:
                        po_tiles[(tq, hc)] = ops.tile([128, 512], f32,
                                                      tag="po", name="po")
                    po = po_tiles[(tq, hc)]
                    for ko in range(8 * r, 8 * r + 8):
                        nc.tensor.matmul(po[:],
                                         wo_sb[:, ko, hc * 128:(hc + 1) * 128],
                                         rt[:, ko, :],
                                         start=(ko == 0), stop=(ko == KO - 1))

                def oproj_fin(tq, hc):
                    po = po_tiles.pop((tq, hc))
                    ot = osb.tile([128, 512], f32, tag="ot")
                    nc.vector.tensor_copy(ot[:], po[:])
                    nc.sync.dma_start(
                        outT.ap()[hc * 128:(hc + 1) * 128,
                                  tq * 512:(tq + 1) * 512],
                        ot[:])

                def oproj_chunks(tq):
                    cls = [lambda tq=tq: oproj_load(tq)]
                    for hc in range(4):
                        for r in range(4):
                            cls.append(lambda tq=tq, hc=hc, r=r:
                                       oproj_run(tq, hc, r))
                        cls.append(lambda tq=tq, hc=hc: oproj_fin(tq, hc))
                    return cls

                fillers = deque(oproj_chunks(0) + oproj_chunks(1))

                # block 3: dense (ACT-bound); a filler o_proj run every few
                # steps soaks the small PE slack without delaying the exps.
                b3 = block_closures(3)
                fillers.popleft()()          # rt0 load: gather 0 is long done
                for i, cl in enumerate(b3):
                    cl()
                    if i % 8 == 7 and fillers:
                        fillers.popleft()()
                while fillers:
                    fillers.popleft()()

                for cl in oproj_chunks(2):
                    cl()
                for cl in oproj_chunks(3):
                    cl()

    nc.compile()
    return nc


def _prep_inputs(positions, hidden_states, w_qkv, w_o, q_norm_w, k_norm_w):
    hidden_states = np.asarray(hidden_states, dtype=np.float32)
    w_qkv = np.asarray(w_qkv, dtype=np.float32)
    w_o = np.asarray(w_o, dtype=np.float32)
    q_norm_w = np.asarray(q_norm_w, dtype=np.float32)
    k_norm_w = np.asarray(k_norm_w, dtype=np.float32)
    pos = np.asarray(positions).astype(np.float32)

    # hiddenT tiled for contiguous per-partition DMA: [tt, p, ko, tl]
    hT = np.ascontiguousarray(
        hidden_states.reshape(TT, 128, KO, 128).transpose(0, 3, 2, 1)
    ).astype(BF16)

    inv_freq = THETA ** (-np.arange(64, dtype=np.float32) / 64.0)
    freqs = pos[:, None] * inv_freq[None, :]
    cos = np.cos(freqs).astype(np.float32).reshape(TT, 128, 64).transpose(1, 0, 2)
    sin = np.sin(freqs).astype(np.float32).reshape(TT, 128, 64).transpose(1, 0, 2)
    cos = np.ascontiguousarray(cos)
    sin = np.ascontiguousarray(sin)

    triu = np.triu(np.ones((128, 128), dtype=np.float32)).astype(BF16)
    identm = np.eye(128, dtype=np.float32).astype(BF16)
    onesm = np.ones((128, 128), dtype=np.float32).astype(BF16)

    in_maps = []
    for c in range(N_CORES):
        qcols = w_qkv[:, 4 * c * HD:(4 * c + 4) * HD]
        kcols = w_qkv[:, NH * HD + c * HD: NH * HD + (c + 1) * HD]
        vcols = w_qkv[:, (NH + NKV) * HD + c * HD: (NH + NKV) * HD + (c + 1) * HD]
        wqkv_sh = np.concatenate([qcols, kcols, vcols], axis=1).astype(BF16)
        wo_sh = np.ascontiguousarray(w_o[:, 512 * c:512 * (c + 1)]).astype(BF16)
        ln5 = np.concatenate([q_norm_w[4 * c:4 * c + 4], k_norm_w[c:c + 1]], axis=0)
        lnw_rep = np.ascontiguousarray(
            np.broadcast_to(ln5[None, :, :], (128, LNH, 128))).astype(np.float32)
        in_maps.append({
            "hT": hT,
            "wqkv": wqkv_sh,
            "wo": wo_sh,
            "cosd": cos,
            "sind": sin,
            "lnw": lnw_rep,
            "triu": triu,
            "ident": identm,
            "onesd": onesm,
        })
    return in_maps


def kernel(positions, hidden_states, w_qkv, w_o, q_norm_w, k_norm_w):
    from concourse.bass_utils import run_bass_kernel_spmd

    if "nc" not in _CACHE:
        _CACHE["nc"] = _build()
    nc = _CACHE["nc"]

    in_maps = _prep_inputs(positions, hidden_states, w_qkv, w_o,
                           q_norm_w, k_norm_w)
    res = run_bass_kernel_spmd(nc, in_maps, core_ids=list(range(N_CORES)))
    out = np.empty((T, H), dtype=np.float32)
    for c in range(N_CORES):
        out[:, 512 * c:512 * (c + 1)] = res.results[c]["outT"].T
    return out
